# revision 1
# baseline (speedup 1.0000x reference)
"""GAT node-classification kernel for Trainium2 (8 NeuronCores, SPMD).

Strategy (dst-node graph partitioning per the sharding hint):
  - Only destination nodes appearing in `ids` affect the output; edges into
    other nodes are dead code and are dropped.
  - Surviving edges are grouped by destination into padded per-slot lists
    (D_PAD edges per slot). High-degree nodes are split over several slots;
    those land in the last K_M tiles of every core and are re-combined on
    device with a 0/1 merge matmul. All other tiles map slot r -> output
    row r directly and skip the merge.
  - Slots are packed into tiles of 128 (partition dim); tiles are sharded
    across the 8 cores (identical program, different data per core).
  - The a_src projection of the gathered neighbour features runs on the
    TensorEngine as xgT.T @ AsBig, where AsBig is a (j-diagonal x As[f,h])
    block matrix built on device. a_dst likewise (xslotT.T @ Ad). The
    leaky-relu/exp/denominator/messages run on Vector/Scalar/GpSimd; the
    weighted message sums stay in the rank-7 feature basis
    (sum(alpha*(x@W)) == (sum(alpha*x))@W); per tile the GAT output,
    LayerNorm, classifier and softmax are dense 128x128 tile math.
  - Max-subtraction is skipped in both softmaxes (bounded fp32 logits;
    ratios unchanged). LayerNorm's affine transform is folded into the
    classifier weights; 1/std is folded through the classifier matmul;
    rstd comes from exp(-0.5*ln(var)) so every activation shares one
    hardware function table.

The host does no floating-point arithmetic on tensor values: it only
filters/sorts/permutes (sharding + halo-exchange layout) and builds 0/1 or
0/-1e30 masks; all float math runs on the NeuronCores.
"""

import os
import sys

sys.path.insert(0, "/opt/trn_rl_repo")

import numpy as np

import concourse.bass as bass
import concourse.bacc as bacc
import concourse.mybir as mybir
import concourse.tile as tile
from concourse import bass_utils
import concourse.bacc as _bacc_mod
import concourse.hw_specs as _hw_specs

_PIN_SET = "natural_log_exp_and_others"
_orig_get_tables = _hw_specs.get_activation_tables


def _pinned_tables(arch):
    """Route every activation to one table set (exp/ln/square/copy all
    coexist there) so the kernel pays a single ACT_TABLE_LOAD."""
    tabs = _orig_get_tables(arch)
    if _PIN_SET in tabs:
        tabs = {k: (v if k == _PIN_SET else set()) for k, v in tabs.items()}
    return tabs


_bacc_mod.get_activation_tables = _pinned_tables

N = 100000
FIN = 7
H = 4
C = 32
HC = H * C  # 128
CLS = 7
NEG = 0.2
D_PAD = 18  # edge slots per node-slot (FIN*D_PAD = 126 <= 128: 1 matmul)
G = 12      # tiles per processing group
NCORES = 8
HF = H * FIN  # 28
F32 = mybir.dt.float32
TJF = D_PAD * FIN  # 126
TJH = D_PAD * H    # 72


# ---------------------------------------------------------------- host prep
def _pack_tiles(node_list, nslot, cnt, starts):
    """Pack nodes' slots into tiles of <=128 slots, no node straddling a
    tile boundary. Returns (tiles, tile_rows); tiles entries are
    (node, first_edge, nedges)."""
    tiles, tile_rows = [], []
    cur_slots, cur_rows = [], []
    for n in node_list:
        ns = int(nslot[n])
        if len(cur_slots) + ns > 128:
            tiles.append(cur_slots)
            tile_rows.append(cur_rows)
            cur_slots, cur_rows = [], []
        e0 = int(starts[n])
        cn = int(cnt[n])
        cur_rows.append(n)
        for k in range(ns):
            a = e0 + k * D_PAD
            b = min(e0 + (k + 1) * D_PAD, e0 + cn)
            cur_slots.append((n, a, max(0, b - a)))
    if cur_slots:
        tiles.append(cur_slots)
        tile_rows.append(cur_rows)
    return tiles, tile_rows


def _preprocess(x, edge_index, ids):
    src = np.asarray(edge_index[0], dtype=np.int64)
    dst = np.asarray(edge_index[1], dtype=np.int64)
    ids = np.asarray(ids, dtype=np.int64)

    uids, inv = np.unique(ids, return_inverse=True)
    U = uids.shape[0]
    mark = np.full(N, -1, np.int64)
    mark[uids] = np.arange(U)

    dstc = mark[dst]
    keep = dstc >= 0
    es = src[keep]
    ed = dstc[keep]
    order = np.argsort(ed, kind="stable")
    es = es[order]
    ed = ed[order]
    cnt = np.bincount(ed, minlength=U).astype(np.int64)
    starts = np.zeros(U + 1, np.int64)
    np.cumsum(cnt, out=starts[1:])

    nslot = np.maximum(1, -(-cnt // D_PAD))
    plain_nodes = np.nonzero(nslot == 1)[0]
    split_nodes = np.nonzero(nslot > 1)[0]

    p_tiles, p_rows = _pack_tiles(plain_nodes, nslot, cnt, starts)
    m_tiles, m_rows = _pack_tiles(split_nodes, nslot, cnt, starts)

    K_M = max(1, -(-len(m_tiles) // NCORES))
    P_pc = -(-len(p_tiles) // NCORES)
    T_pc = -(-(K_M + P_pc) // G) * G
    P_pc = T_pc - K_M
    T_tot = T_pc * NCORES

    src_pad = np.zeros((T_tot, 128, D_PAD), np.int64)
    deg = np.zeros((T_tot, 128), np.int64)
    slotnode = np.zeros((T_tot, 128), np.int64)
    mergeT = np.tile(np.eye(128, dtype=np.float32), (NCORES, K_M, 1, 1))
    row_node = np.full((T_tot, 128), -1, np.int64)

    # plain tiles: core c, tiles [c*T_pc, c*T_pc + P_pc); row == slot
    for i, (slots, rows) in enumerate(zip(p_tiles, p_rows)):
        c, k = divmod(i, P_pc)
        gt = c * T_pc + k
        for s, (n, a, ln) in enumerate(slots):
            slotnode[gt, s] = uids[n]
            deg[gt, s] = ln
            if ln > 0:
                src_pad[gt, s, :ln] = es[a:a + ln]
            row_node[gt, s] = n

    # split tiles: core c, tiles [c*T_pc + P_pc, (c+1)*T_pc)
    for i, (slots, rows) in enumerate(zip(m_tiles, m_rows)):
        c, k = divmod(i, K_M)
        gt = c * T_pc + P_pc + k
        mergeT[c, k] = 0.0
        rpos = {n: r for r, n in enumerate(rows)}
        for s, (n, a, ln) in enumerate(slots):
            slotnode[gt, s] = uids[n]
            deg[gt, s] = ln
            if ln > 0:
                src_pad[gt, s, :ln] = es[a:a + ln]
            mergeT[c, k, s, rpos[n]] = 1.0
        for r, n in enumerate(rows):
            row_node[gt, r] = n

    xg4 = x[src_pad.reshape(-1)].reshape(T_tot, 128, D_PAD, FIN)
    xg = np.ascontiguousarray(xg4.transpose(0, 1, 3, 2))  # [T,128,(f,j)]
    xgT = np.ascontiguousarray(xg4.transpose(0, 3, 2, 1)).reshape(
        T_tot, TJF, 128)  # rows (f, j)
    xslotT = np.ascontiguousarray(
        x[slotnode.reshape(-1)].reshape(T_tot, 128, FIN).transpose(0, 2, 1))
    j = np.arange(D_PAD)[None, None, :]
    maskB = np.where(j < deg[:, :, None], 0.0, -1e30).astype(np.float32)

    # jdiag [(f,jj), (j',h)] = (j' == jj)
    eqjj = np.arange(D_PAD)[None, :] == np.arange(D_PAD)[:, None]
    jd = np.tile(eqjj.astype(np.float32).reshape(1, D_PAD, D_PAD, 1),
                 (FIN, 1, 1, H)).reshape(TJF, TJH)

    rows_flat = row_node.reshape(-1)
    out_row_of_node = np.zeros(U, np.int64)
    valid = rows_flat >= 0
    out_row_of_node[rows_flat[valid]] = np.nonzero(valid)[0]

    return {
        "T_pc": T_pc,
        "K_M": K_M,
        "xg": xg.reshape(T_tot, 128, TJF).astype(np.float32, copy=False),
        "xgT": xgT.astype(np.float32, copy=False),
        "xslotT": xslotT.astype(np.float32, copy=False),
        "jd": jd,
        "maskB": maskB,
        "mergeT": mergeT,
        "out_row_of_node": out_row_of_node,
        "inv": inv,
    }


def _ap(base, off_elems, dims):
    """AP with explicit free dims; dims = [[step, count], ...]."""
    return bass.AP(base.tensor, base.offset + off_elems, [list(base.ap[0])] + dims)


# ---------------------------------------------------------------- program
def _build(T_pc, K_M):
    nc = bacc.Bacc("TRN2", target_bir_lowering=False, debug=False,
                   num_devices=NCORES)
    NG = T_pc // G
    P_pc = T_pc - K_M

    d_xg = nc.dram_tensor("xg", [T_pc, 128, TJF], F32, kind="ExternalInput")
    d_xgT = nc.dram_tensor("xgT", [T_pc, TJF, 128], F32, kind="ExternalInput")
    d_xslT = nc.dram_tensor("xslotT", [T_pc, FIN, 128], F32, kind="ExternalInput")
    d_jd = nc.dram_tensor("jd", [TJF, TJH], F32, kind="ExternalInput")
    d_mB = nc.dram_tensor("maskB", [T_pc, 128, D_PAD], F32, kind="ExternalInput")
    d_mg = nc.dram_tensor("mergeT", [K_M, 128, 128], F32, kind="ExternalInput")
    d_W = nc.dram_tensor("W", [FIN, HC], F32, kind="ExternalInput")
    d_attS = nc.dram_tensor("attS", [1, HC], F32, kind="ExternalInput")
    d_attD = nc.dram_tensor("attD", [1, HC], F32, kind="ExternalInput")
    d_gb = nc.dram_tensor("gbias", [1, HC], F32, kind="ExternalInput")
    d_lnw = nc.dram_tensor("lnw", [1, HC], F32, kind="ExternalInput")
    d_lnb = nc.dram_tensor("lnb", [1, HC], F32, kind="ExternalInput")
    d_lW = nc.dram_tensor("linW", [HC, CLS], F32, kind="ExternalInput")
    d_lb = nc.dram_tensor("linb", [1, CLS], F32, kind="ExternalInput")
    d_id = nc.dram_tensor("ident", [128, 128], F32, kind="ExternalInput")
    d_out = nc.dram_tensor("probs", [T_pc * 128, CLS], F32, kind="ExternalOutput")
    d_scr = nc.dram_tensor("scratch", [3, HF], F32, kind="ExternalOutput")

    AX = mybir.AxisListType.X
    OP = mybir.AluOpType
    ACT = mybir.ActivationFunctionType

    with tile.TileContext(nc) as tc:
        with (
            tc.tile_pool(name="const", bufs=1) as cp,
            tc.tile_pool(name="work", bufs=2) as wp,
            tc.tile_pool(name="psum", bufs=1, space="PSUM") as pp,
            tc.tile_pool(name="psum2", bufs=2, space="PSUM") as pp2,
        ):
            # ---- prologue: constants
            W_sb = cp.tile([FIN, HC], F32, tag="W")
            nc.sync.dma_start(out=W_sb[:], in_=d_W[:, :])
            ident = cp.tile([128, 128], F32, tag="ident")
            nc.sync.dma_start(out=ident[:], in_=d_id[:, :])

            # As/Ad [7,4] = per-head dot of W rows with attention vectors
            AsAd = []
            for nm, drow in (("S", d_attS), ("D", d_attD)):
                att_b = cp.tile([FIN, HC], F32, tag=f"att{nm}b")
                nc.sync.dma_start(out=att_b[:],
                                  in_=drow[0:1, :].to_broadcast([FIN, HC]))
                tmp = cp.tile([FIN, HC], F32, tag=f"att{nm}t")
                nc.vector.tensor_tensor(out=tmp[:], in0=W_sb[:], in1=att_b[:],
                                        op=OP.mult)
                a74 = cp.tile([FIN, H], F32, tag=f"a74{nm}")
                nc.vector.tensor_reduce(
                    out=a74[:], in_=tmp[:].rearrange("p (h c) -> p h c", h=H),
                    axis=AX, op=OP.add)
                AsAd.append(a74)
            nc.sync.dma_start(out=d_scr[0:1, :], in_=AsAd[0][:, :])

            # AsBig [(f,jj), (j',h)] = As[f,h] * (j'==jj)
            As_pat = cp.tile([TJF, TJH], F32, tag="Aspat")
            for f in range(FIN):
                dst = As_pat[f * D_PAD:(f + 1) * D_PAD, :]
                dst = bass.AP(dst.tensor, dst.offset,
                              [list(dst.ap[0]), [H, D_PAD], [1, H]])
                srcap = d_scr[0:1, :]
                srcap = bass.AP(srcap.tensor, srcap.offset + f * H,
                                [[0, D_PAD], [0, D_PAD], [1, H]])
                nc.sync.dma_start(out=dst, in_=srcap)
            jd_sb = cp.tile([TJF, TJH], F32, tag="jd")
            nc.sync.dma_start(out=jd_sb[:], in_=d_jd[:, :])
            AsBig = cp.tile([TJF, TJH], F32, tag="AsBig")
            nc.gpsimd.tensor_tensor(out=AsBig[:], in0=As_pat[:], in1=jd_sb[:],
                                    op=OP.mult)

            # Wb [28,128] block-diagonal W
            Wb = cp.tile([HF, HC], F32, tag="Wb")
            nc.gpsimd.memset(Wb[:], 0.0)
            for h in range(H):
                nc.sync.dma_start(out=Wb[h * FIN:(h + 1) * FIN, h * C:(h + 1) * C],
                                  in_=d_W[:, h * C:(h + 1) * C])

            eps_c = cp.tile([128, 1], F32, tag="epsc")
            nc.gpsimd.memset(eps_c[:], 1e-5)

            # gbc = gat_bias - mean(gat_bias), broadcast [128, HC]
            gb_bc = cp.tile([128, HC], F32, tag="gbbc")
            nc.sync.dma_start(out=gb_bc[:], in_=d_gb[0:1, :].to_broadcast([128, HC]))
            gbm = cp.tile([128, 1], F32, tag="gbm")
            nc.vector.tensor_reduce(out=gbm[:], in_=gb_bc[:], axis=AX, op=OP.add)
            nc.vector.tensor_scalar(out=gbm[:], in0=gbm[:], scalar1=1.0 / HC,
                                    scalar2=None, op0=OP.mult)
            gbc_bc = cp.tile([128, HC], F32, tag="gbcbc")
            nc.vector.tensor_scalar(out=gbc_bc[:], in0=gb_bc[:], scalar1=gbm[:, 0:1],
                                    scalar2=None, op0=OP.subtract)

            # linW' = lnw-scaled linW ; lb' = lnb @ linW + lb
            lnw_col = cp.tile([HC, 1], F32, tag="lnwcol")
            nc.sync.dma_start(out=lnw_col[:], in_=d_lnw[0:1, :].rearrange("o e -> e o"))
            lnb_col = cp.tile([HC, 1], F32, tag="lnbcol")
            nc.sync.dma_start(out=lnb_col[:], in_=d_lnb[0:1, :].rearrange("o e -> e o"))
            linW = cp.tile([HC, CLS], F32, tag="linW")
            nc.sync.dma_start(out=linW[:], in_=d_lW[:, :])
            linWp = cp.tile([HC, CLS], F32, tag="linWp")
            nc.vector.tensor_scalar(out=linWp[:], in0=linW[:], scalar1=lnw_col[:, 0:1],
                                    scalar2=None, op0=OP.mult)
            ps_lb = pp.tile([1, CLS], F32, tag="psm")
            nc.tensor.matmul(out=ps_lb[:], lhsT=lnb_col[:], rhs=linW[:],
                             start=True, stop=True)
            lb_sb = cp.tile([1, CLS], F32, tag="lbrow")
            nc.sync.dma_start(out=lb_sb[:], in_=d_lb[0:1, :])
            lbp_sb = cp.tile([1, CLS], F32, tag="lbp")
            nc.vector.tensor_tensor(out=lbp_sb[:], in0=ps_lb[:], in1=lb_sb[:],
                                    op=OP.add)
            nc.sync.dma_start(out=d_scr[2:3, 0:CLS], in_=lbp_sb[:])
            lbp_bc = cp.tile([128, CLS], F32, tag="lbpbc")
            nc.sync.dma_start(out=lbp_bc[:],
                              in_=d_scr[2:3, 0:CLS].to_broadcast([128, CLS]))

            # ad via PE: psum_ad[:, t*4:(t+1)*4] = xslT_t.T @ Ad74
            xslT = cp.tile([FIN, T_pc * 128], F32, tag="xslT")
            nc.sync.dma_start(
                out=xslT[:].rearrange("p (t s) -> p t s", t=T_pc),
                in_=d_xslT[:, :, :].rearrange("t p s -> p t s"))
            ps_ad = pp.tile([128, T_pc * H], F32, tag="psas")
            for t in range(T_pc):
                nc.tensor.matmul(
                    out=ps_ad[:, t * H:(t + 1) * H],
                    lhsT=xslT[:, t * 128:(t + 1) * 128],
                    rhs=AsAd[1][:], start=True, stop=True)
            # adB[p, (t,j,h)] = ad[t,h] + maskB[t,j]
            mB_all = cp.tile([128, T_pc * D_PAD], F32, tag="mBall")
            nc.sync.dma_start(
                out=mB_all[:].rearrange("p (t e) -> p t e", t=T_pc),
                in_=d_mB[:, :, :].rearrange("t p e -> p t e"))
            adB = cp.tile([128, T_pc * TJH], F32, tag="adB")
            nc.vector.tensor_tensor(
                out=adB[:],
                in0=_ap(ps_ad[:], 0, [[H, T_pc], [0, D_PAD], [1, H]]),
                in1=_ap(mB_all[:], 0, [[D_PAD, T_pc], [1, D_PAD], [0, H]]),
                op=OP.add)

            # merge matrices for the K_M merge tiles
            mg_sb = cp.tile([128, K_M * 128], F32, tag="mg")
            nc.sync.dma_start(
                out=mg_sb[:].rearrange("p (t e) -> p t e", t=K_M),
                in_=d_mg[:, :, :].rearrange("t p e -> p t e"))

            # ---- main loop over groups of G tiles
            for g in range(NG):
                t0 = g * G
                xg_sb = wp.tile([128, G * TJF], F32, tag="xg")
                nc.sync.dma_start(
                    out=xg_sb[:].rearrange("p (t e) -> p t e", t=G),
                    in_=d_xg[t0:t0 + G, :, :].rearrange("t p e -> p t e"))
                xgT_sb = wp.tile([TJF, G * 128], F32, tag="xgT")
                nc.sync.dma_start(
                    out=xgT_sb[:].rearrange("p (t s) -> p t s", t=G),
                    in_=d_xgT[t0:t0 + G, :, :].rearrange("t p s -> p t s"))

                xgb = xg_sb[:]
                # as[p, (t,j,h)] via PE: xgT.T @ AsBig
                ps_as = pp.tile([128, G * 128], F32, tag="psas")
                for t in range(G):
                    nc.tensor.matmul(
                        out=ps_as[:, t * 128:t * 128 + TJH],
                        lhsT=xgT_sb[:, t * 128:(t + 1) * 128],
                        rhs=AsBig[:], start=True, stop=True)
                # s = as + adB ; ez = max(NEG*s, s) ;
                # expz = exp(ez), relayout (t,j,h) -> (t,h,j) inside the ACT
                s_sb = wp.tile([128, G * TJH], F32, tag="s")
                nc.vector.tensor_tensor(
                    out=_ap(s_sb[:], 0, [[TJH, G], [1, TJH]]),
                    in0=_ap(ps_as[:], 0, [[128, G], [1, TJH]]),
                    in1=_ap(adB[:], t0 * TJH, [[TJH, G], [1, TJH]]), op=OP.add)
                ez_sb = wp.tile([128, G * TJH], F32, tag="ez")
                nc.vector.scalar_tensor_tensor(
                    out=ez_sb[:], in0=s_sb[:], scalar=NEG, in1=s_sb[:],
                    op0=OP.mult, op1=OP.max)
                ez2 = wp.tile([128, G * TJH], F32, tag="ez2")  # (t, h, j)
                nc.scalar.activation(
                    out=_ap(ez2[:], 0, [[TJH, G], [D_PAD, H], [1, D_PAD]]),
                    in_=_ap(ez_sb[:], 0, [[TJH, G], [1, H], [H, D_PAD]]),
                    func=ACT.Exp)

                # SD[p, t*32+(h*7+f)] = sum_j ez*xg ; SD[p, t*32+28+h] = denom
                SD = wp.tile([128, G * 32], F32, tag="SD")
                nc.vector.tensor_reduce(
                    out=_ap(SD[:], 28, [[32, G], [1, H]]),
                    in_=_ap(ez2[:], 0, [[TJH, G], [D_PAD, H], [1, D_PAD]]),
                    axis=AX, op=OP.add)
                for t in range(G):
                    prod = wp.tile([128, D_PAD * HF], F32, tag="prod")
                    ez_t = _ap(ez2[:], t * TJH, [[D_PAD, H], [0, FIN], [1, D_PAD]])
                    xg_t = _ap(xgb, t * TJF, [[0, H], [D_PAD, FIN], [1, D_PAD]])
                    nc.gpsimd.tensor_tensor(out=prod[:], in0=ez_t, in1=xg_t,
                                            op=OP.mult)
                    nc.vector.tensor_reduce(
                        out=SD[:, t * 32:t * 32 + HF],
                        in_=prod[:].rearrange("p (hf j) -> p hf j", j=D_PAD),
                        axis=AX, op=OP.add)

                # merge matmul only for the last K_M tiles of the core
                ps_m = pp.tile([128, G * 32], F32, tag="psm")
                merged = {}
                for t in range(G):
                    gt = t0 + t
                    if gt >= P_pc:
                        km = gt - P_pc
                        nc.tensor.matmul(
                            out=ps_m[:, t * 32:(t + 1) * 32],
                            lhsT=mg_sb[:, km * 128:(km + 1) * 128],
                            rhs=SD[:, t * 32:(t + 1) * 32],
                            start=True, stop=True)
                        merged[t] = True

                def sd_ap(t, lo, hi):
                    srcbuf = ps_m if merged.get(t) else SD
                    return srcbuf[:, t * 32 + lo:t * 32 + hi]

                rd = wp.tile([128, G * H], F32, tag="rd")
                for t in range(G):
                    nc.vector.tensor_scalar(
                        out=rd[:, t * H:(t + 1) * H], in0=sd_ap(t, 28, 32),
                        scalar1=1e-16, scalar2=None, op0=OP.add)
                nc.vector.reciprocal(out=rd[:], in_=rd[:])
                Sn = wp.tile([128, G * HF], F32, tag="Sn")
                for t in range(G):
                    base = sd_ap(t, 0, HF)
                    nc.vector.tensor_tensor(
                        out=_ap(Sn[:], t * HF, [[FIN, H], [1, FIN]]),
                        in0=bass.AP(base.tensor, base.offset,
                                    [list(base.ap[0]), [FIN, H], [1, FIN]]),
                        in1=_ap(rd[:], t * H, [[1, H], [0, FIN]]),
                        op=OP.mult)

                # per tile: SnT = transpose(Sn_t); out128 = SnT.T @ Wb
                mu = wp.tile([128, G], F32, tag="mu")
                vs = wp.tile([128, G], F32, tag="vs")
                c_sb = wp.tile([128, G * HC], F32, tag="c")
                lg = wp.tile([128, G * CLS], F32, tag="lg")
                for t in range(G):
                    ps_tT = pp2.tile([HF, 128], F32, tag="pst")
                    nc.tensor.transpose(out=ps_tT[:],
                                        in_=Sn[:, t * HF:(t + 1) * HF],
                                        identity=ident[:])
                    SnT = wp.tile([HF, 128], F32, tag="SnT")
                    nc.scalar.copy(out=SnT[:], in_=ps_tT[:])
                    ps_o = pp2.tile([128, HC], F32, tag="pso")
                    nc.tensor.matmul(
                        out=ps_o[:], lhsT=SnT[:],
                        rhs=Wb[:], start=True, stop=True)
                    # LayerNorm: c = (o - mu) + gbc
                    nc.vector.tensor_reduce(
                        out=mu[:, t:t + 1], in_=ps_o[:], axis=AX, op=OP.add)
                    nc.vector.tensor_scalar(
                        out=mu[:, t:t + 1], in0=mu[:, t:t + 1],
                        scalar1=1.0 / HC, scalar2=None, op0=OP.mult)
                    nc.vector.scalar_tensor_tensor(
                        out=c_sb[:, t * HC:(t + 1) * HC], in0=ps_o[:],
                        scalar=mu[:, t:t + 1], in1=gbc_bc[:],
                        op0=OP.subtract, op1=OP.add)
                    sqtmp = wp.tile([128, HC], F32, tag="sqtmp")
                    nc.scalar.activation(out=sqtmp[:],
                                         in_=c_sb[:, t * HC:(t + 1) * HC],
                                         func=ACT.Square,
                                         accum_out=vs[:, t:t + 1])
                nc.scalar.activation(out=vs[:], in_=vs[:], func=ACT.Ln,
                                     scale=1.0 / HC, bias=eps_c[:, 0:1])
                nc.scalar.activation(out=vs[:], in_=vs[:], func=ACT.Exp,
                                     scale=-0.5)

                # classifier on c; rstd folded in after the matmul
                for t in range(G):
                    ps_t2 = pp2.tile([128, HC], F32, tag="pso")
                    nc.tensor.transpose(out=ps_t2[:],
                                        in_=c_sb[:, t * HC:(t + 1) * HC],
                                        identity=ident[:])
                    onT = wp.tile([128, HC], F32, tag="onT")
                    nc.scalar.copy(out=onT[:], in_=ps_t2[:])
                    ps_l = pp2.tile([128, CLS], F32, tag="pst")
                    nc.tensor.matmul(
                        out=ps_l[:], lhsT=onT[:],
                        rhs=linWp[:], start=True, stop=True)
                    nc.vector.scalar_tensor_tensor(
                        out=lg[:, t * CLS:(t + 1) * CLS],
                        in0=ps_l[:],
                        scalar=vs[:, t:t + 1], in1=lbp_bc[:],
                        op0=OP.mult, op1=OP.add)
                nc.scalar.activation(out=lg[:], in_=lg[:], func=ACT.Exp)
                se = wp.tile([128, G], F32, tag="se")
                nc.vector.tensor_reduce(
                    out=se[:], in_=lg[:].rearrange("p (t e) -> p t e", t=G),
                    axis=AX, op=OP.add)
                nc.vector.reciprocal(out=se[:], in_=se[:])
                pr_out = wp.tile([128, G * CLS], F32, tag="prout")
                nc.gpsimd.tensor_tensor(
                    out=pr_out[:],
                    in0=lg[:].rearrange("p (t e) -> p t e", t=G),
                    in1=_ap(se[:], 0, [[1, G], [0, CLS]]), op=OP.mult)

                nc.sync.dma_start(
                    out=d_out[t0 * 128:(t0 + G) * 128, :].rearrange(
                        "(t p) c -> p t c", t=G),
                    in_=pr_out[:].rearrange("p (t c) -> p t c", t=G))

    nc.compile()
    return nc


_CACHE = {}


def _program(T_pc, K_M):
    key = (T_pc, K_M)
    if key not in _CACHE:
        _CACHE[key] = _build(T_pc, K_M)
    return _CACHE[key]


# ---------------------------------------------------------------- entry
def kernel(x, edge_weight, W, att_src, att_dst, gat_bias, ln_w, ln_b,
           lin_W, lin_b, edge_index, ids):
    x = np.asarray(x, np.float32)
    prep = _preprocess(x, np.asarray(edge_index), np.asarray(ids))
    T_pc = prep["T_pc"]
    K_M = prep["K_M"]
    nc = _program(T_pc, K_M)

    shared = {
        "W": np.ascontiguousarray(W, np.float32).reshape(FIN, HC),
        "attS": np.ascontiguousarray(att_src, np.float32).reshape(1, HC),
        "attD": np.ascontiguousarray(att_dst, np.float32).reshape(1, HC),
        "gbias": np.ascontiguousarray(gat_bias, np.float32).reshape(1, HC),
        "lnw": np.ascontiguousarray(ln_w, np.float32).reshape(1, HC),
        "lnb": np.ascontiguousarray(ln_b, np.float32).reshape(1, HC),
        "linW": np.ascontiguousarray(lin_W, np.float32).reshape(HC, CLS),
        "linb": np.ascontiguousarray(lin_b, np.float32).reshape(1, CLS),
        "ident": np.eye(128, dtype=np.float32),
        "jd": prep["jd"],
    }
    in_maps = []
    for c in range(NCORES):
        sl = slice(c * T_pc, (c + 1) * T_pc)
        in_maps.append({
            "xg": prep["xg"][sl],
            "xgT": prep["xgT"][sl],
            "xslotT": prep["xslotT"][sl],
            "maskB": prep["maskB"][sl],
            "mergeT": prep["mergeT"][c],
            **shared,
        })

    if os.environ.get("KERNEL_SIM"):
        from concourse.bass_interp import CoreSim

        outs = []
        ncores = int(os.environ.get("KERNEL_SIM_CORES", "1"))
        for c in range(ncores):
            sim = CoreSim(nc, require_finite=False, require_nnan=False)
            for k, v in in_maps[c].items():
                sim.tensor(k)[:] = v
            sim.simulate()
            outs.append(sim.tensor("probs").copy())
        full = np.concatenate(
            outs + [np.zeros_like(outs[0])] * (NCORES - ncores), 0)
        probs_u = full[prep["out_row_of_node"]]
        return np.ascontiguousarray(probs_u[prep["inv"]], np.float32)

    trace = bool(int(os.environ.get("KERNEL_TRACE", "0")))
    res = bass_utils.run_bass_kernel_spmd(
        nc, in_maps, core_ids=list(range(NCORES)), trace=trace)
    if trace and res.exec_time_ns is not None:
        print(f"HW exec time: {res.exec_time_ns} ns")

    full = np.concatenate([res.results[c]["probs"] for c in range(NCORES)], 0)
    probs_u = full[prep["out_row_of_node"]]
    return np.ascontiguousarray(probs_u[prep["inv"]], np.float32)



# revision 21
# speedup vs baseline: 1.5654x; 1.5654x over previous
"""GAT node-classification kernel for Trainium2 (8 NeuronCores, SPMD).

Strategy (dst-node graph partitioning per the sharding hint):
  - Only destination nodes appearing in `ids` affect the output; edges into
    other nodes are dead code and are dropped.
  - D_PAD is the max surviving in-degree (33 here), so every destination
    node owns exactly one 128-slot row: no split nodes, no merge matmuls.
    An extra pseudo-neighbor column (j = D_PAD) carries the node's own
    features so a_dst falls out of the same contraction machinery.
  - Per-edge attention logits are 7-term feature contractions computed on
    the Vector engine in bf16 (4x DVE mode: all operands 2-byte, packed,
    SBUF); reductions are batched binary trees of tensor_tensor adds (the
    last levels accumulate in fp32).  Weighted message sums stay in the
    rank-7 feature basis (sum(alpha*(x@W)) == (sum(alpha*x))@W).
  - Per tile the normalized sums (plus an appended ones column that carries
    the GAT bias and folded classifier bias) are transposed once on the PE
    (4 tiles per 128-wide transpose, quadrant-aligned) and hit a single
    fp32r matmul with rhs = [Wb | Wb@linWp' | -rowsum/HC], yielding the GAT
    output o, the classifier projection q, and -mean(o) in one pass.
    LayerNorm's affine transform is folded into the classifier weights;
    1/std is applied after the matmul; rstd = exp(-0.5*ln(var)).
  - Max-subtraction is skipped in both softmaxes (bounded logits).
  - All device inputs are packed host-side so each SBUF partition's data is
    contiguous in DRAM (one descriptor set per transfer, large packets).

The host does no floating-point arithmetic on tensor values: it only
filters/sorts/permutes (sharding layout, gathers, block-diagonal placement
of W) and builds 0/1, eye, ones and 0/-1e30 masks; all float math and all
dtype conversion runs on the NeuronCores.
"""

import os
import sys

sys.path.insert(0, "/opt/trn_rl_repo")

import numpy as np

import concourse.bass as bass
import concourse.bacc as bacc
import concourse.mybir as mybir
import concourse.tile as tile
from concourse import bass_utils
import concourse.bacc as _bacc_mod
import concourse.hw_specs as _hw_specs

_PIN_SET = "natural_log_exp_and_others"
_orig_get_tables = _hw_specs.get_activation_tables


def _pinned_tables(arch):
    """Route every activation to one table set (exp/ln/square/copy all
    coexist there) so the kernel pays a single ACT_TABLE_LOAD."""
    tabs = _orig_get_tables(arch)
    if _PIN_SET in tabs:
        tabs = {k: (v if k == _PIN_SET else set()) for k, v in tabs.items()}
    return tabs


_bacc_mod.get_activation_tables = _pinned_tables

N = 100000
FIN = 7
H = 4
C = 32
HC = H * C  # 128
CLS = 7
NEG = 0.2
NCORES = 8
F32 = mybir.dt.float32
F32R = mybir.dt.float32r
F16 = mybir.dt.float16

# const-pack column offsets (cursor-built)
_cur = 0


def _adv(w):
    global _cur
    o = _cur
    _cur += w
    return o


C_ID = _adv(128)     # identity [128,128]
C_WB = _adv(136)     # [Wb;gb] quadrant-replicated (128) | Wbc (7) | -rowsum/HC
C_WT = _adv(128)     # WT4G [128,125] (pad 3)
C_LIN = _adv(14)     # linW | (linWp written by device)
C_LNB = _adv(1)      # lnb column
C_ONE = _adv(1)      # ones column (adjacent: lhsT [128,2])
C_LNW = _adv(1)      # lnw column
C_EPS = _adv(1)      # 1e-5 column
C_E16 = _adv(1)      # 1e-16 column
C_WD = _adv(256)     # rows 0:7 = [W | W]
C_AT = _adv(256)     # rows 0:7 = [attS bcast | attD bcast]
C_SEL = _adv(14)     # rows 0:2 = keep-mask for [lbp | colsum] assembly
C_LB2 = _adv(14)     # row 0 cols 0:7 = lin_b, else 0
C_O2 = _adv(128)     # rows 0:2 = ones
NC = _cur + (-_cur % 8)


# ---------------------------------------------------------------- host prep
def _preprocess(x, edge_index, ids):
    src = np.asarray(edge_index[0], dtype=np.int64)
    dst = np.asarray(edge_index[1], dtype=np.int64)
    ids = np.asarray(ids, dtype=np.int64)
    x = np.asarray(x, np.float32)

    uids, inv = np.unique(ids, return_inverse=True)
    U = uids.shape[0]
    mark = np.full(N, -1, np.int64)
    mark[uids] = np.arange(U)

    dstc = mark[dst]
    keep = dstc >= 0
    es = src[keep]
    ed = dstc[keep]
    order = np.argsort(ed, kind="stable")
    es = es[order]
    ed = ed[order]
    cnt = np.bincount(ed, minlength=U).astype(np.int64)
    starts = np.zeros(U + 1, np.int64)
    np.cumsum(cnt, out=starts[1:])

    D_PAD = max(1, int(cnt.max()))
    DE = D_PAD + 1            # extra pseudo-neighbor column = own features
    COLT = FIN * DE + DE      # xg (f-major, incl xslot at j=D_PAD) | mask
    T_need = -(-U // 128)
    T_pc = -(-T_need // NCORES)
    T_pc += T_pc % 2          # even, for 2 pipeline groups
    T_tot = T_pc * NCORES
    Upad = T_tot * 128

    cnt_p = np.zeros(Upad, np.int64)
    cnt_p[:U] = cnt
    uids_p = np.zeros(Upad, np.int64)
    uids_p[:U] = uids

    src_pad = np.zeros((Upad, DE), np.int64)
    col_of_edge = np.arange(es.shape[0]) - starts[ed]
    src_pad[ed, col_of_edge] = es
    src_pad[:, D_PAD] = uids_p

    xg2 = np.zeros((Upad, COLT), np.float32)
    xg2[:, : FIN * DE] = (
        x[src_pad.reshape(-1)]
        .reshape(Upad, DE, FIN)
        .transpose(0, 2, 1)
        .reshape(Upad, FIN * DE)
    )
    j = np.arange(DE)[None, :]
    xg2[:, FIN * DE :] = np.where(j < cnt_p[:, None], 0.0, -60000.0
                                  ).astype(np.float32)

    xg2 = np.ascontiguousarray(
        xg2.reshape(NCORES, T_pc, 128, COLT)
        .transpose(0, 2, 1, 3)
        .reshape(NCORES, 128, T_pc * COLT)
    )
    return {"T_pc": T_pc, "D_PAD": D_PAD, "COLT": COLT, "U": U,
            "xg2": xg2, "inv": inv}


def _const_pack(W, att_src, att_dst, gat_bias, ln_w, ln_b, lin_W, lin_b):
    W = np.ascontiguousarray(W, np.float32).reshape(FIN, HC)
    attS = np.ascontiguousarray(att_src, np.float32).reshape(HC)
    attD = np.ascontiguousarray(att_dst, np.float32).reshape(HC)
    gb = np.ascontiguousarray(gat_bias, np.float32).reshape(HC)
    cp = np.zeros((128, NC), np.float32)
    cp[:, C_ID : C_ID + 128] = np.eye(128, dtype=np.float32)
    # WbFull rows 32q+r: r<28 -> Wb row r ((h,f)=divmod(r,7)); r==28 -> gb
    wb = np.zeros((32, 128), np.float32)
    for r in range(28):
        h, f = divmod(r, FIN)
        wb[r, h * C : (h + 1) * C] = W[f, h * C : (h + 1) * C]
    wb[28, :] = gb
    for q in range(4):
        cp[32 * q : 32 * (q + 1), C_WB : C_WB + 128] = wb
    # WT4G col 32q+m: m<28 -> Wb row m transposed; m==28 -> gb
    wt = np.zeros((128, 32), np.float32)
    wt[:, :29] = wb[:29].T
    cp[:, C_WT : C_WT + 125] = np.tile(wt, (1, 4))[:, :125]
    cp[:, C_LIN : C_LIN + CLS] = np.ascontiguousarray(
        lin_W, np.float32).reshape(HC, CLS)
    cp[:, C_LNB] = np.ascontiguousarray(ln_b, np.float32).reshape(HC)
    cp[:, C_ONE] = 1.0
    cp[:, C_LNW] = np.ascontiguousarray(ln_w, np.float32).reshape(HC)
    cp[:, C_EPS] = 1e-5
    cp[:, C_E16] = 1e-16
    cp[0:FIN, C_WD : C_WD + 128] = W
    cp[0:FIN, C_WD + 128 : C_WD + 256] = W
    cp[0:FIN, C_AT : C_AT + 128] = np.tile(attS[None, :], (FIN, 1))
    cp[0:FIN, C_AT + 128 : C_AT + 256] = np.tile(attD[None, :], (FIN, 1))
    cp[0, C_SEL : C_SEL + CLS] = 1.0
    cp[1, C_SEL + CLS : C_SEL + 14] = 1.0
    cp[0, C_LB2 : C_LB2 + CLS] = np.ascontiguousarray(
        lin_b, np.float32).reshape(CLS)
    cp[0:2, C_O2 : C_O2 + 128] = 1.0
    return cp


def _ap(base, off_elems, dims):
    """AP with explicit free dims; dims = [[step, count], ...]."""
    return bass.AP(base.tensor, base.offset + off_elems,
                   [list(base.ap[0])] + dims)


# ---------------------------------------------------------------- program
def _build(T_pc, D_PAD, COLT):
    nc = bacc.Bacc("TRN2", target_bir_lowering=False, debug=False,
                   num_devices=NCORES)
    G = T_pc // 2
    DE = D_PAD + 1
    JF = FIN * DE             # xg cols per tile (incl xslot at j=D_PAD)
    JH = H * DE               # (h,j) logits per tile
    MJF = FIN * D_PAD         # message product cols per tile
    O_MK = JF                 # mask offset within tile cols
    NB = -(-G // 3)           # transpose batches per group (3 tiles/batch:
                              # lhsT base partitions limited to 0/32/64)

    d_xg = nc.dram_tensor("xg2", [128, T_pc * COLT], F32, kind="ExternalInput")
    d_cp = nc.dram_tensor("cpack", [128, NC], F32, kind="ExternalInput")
    d_scr = nc.dram_tensor("scratch", [8, FIN], F32, kind="ExternalOutput")
    d_out = nc.dram_tensor("probs", [128, T_pc * CLS], F32,
                           kind="ExternalOutput")

    AX = mybir.AxisListType.X
    OP = mybir.AluOpType
    ACT = mybir.ActivationFunctionType

    with tile.TileContext(nc) as tc:
        with (
            tc.tile_pool(name="const", bufs=1) as cp,
            tc.tile_pool(name="work", bufs=2) as wp,
            tc.tile_pool(name="pp_p", bufs=1, space="PSUM") as pp_p,
            tc.tile_pool(name="pp_t", bufs=2, space="PSUM") as pp_t,
            tc.tile_pool(name="pp_o", bufs=2, space="PSUM") as pp_o,
        ):
            # ---- prologue: one packed const DMA, then on-device weight prep
            CP = cp.tile([128, NC], F32, tag="CP")
            nc.sync.dma_start(out=CP[:], in_=d_cp[:, :])
            ident = CP[:, C_ID : C_ID + 128]

            # As/Ad [f,(which,h)] = per-head dot of W rows with att vectors
            tmp78 = cp.tile([FIN, 256], F32, tag="t78")
            nc.vector.tensor_tensor(
                out=tmp78[:], in0=CP[0:FIN, C_WD : C_WD + 256],
                in1=CP[0:FIN, C_AT : C_AT + 256], op=OP.mult)
            a78 = cp.tile([FIN, 8], F32, tag="a78")
            nc.vector.tensor_reduce(
                out=_ap(a78[:], 0, [[4, 2], [1, 4]]),
                in_=_ap(tmp78[:], 0, [[128, 2], [32, 4], [1, 32]]),
                axis=AX, op=OP.add)
            # broadcast to all partitions via DRAM roundtrip: [128,(which,h,f)]
            srcp = d_scr[0:1, 0:1]
            nc.sync.dma_start(
                out=bass.AP(srcp.tensor, srcp.offset, [[1, FIN], [FIN, 8]]),
                in_=a78[:])
            AsAd = cp.tile([128, 56], F32, tag="AsAd")
            nc.sync.dma_start(
                out=AsAd[:],
                in_=bass.AP(srcp.tensor, srcp.offset, [[0, 128], [1, 56]]))
            # expanded bf16 (h,f,j) table: As for j<D_PAD, Ad at j=D_PAD
            AsE = cp.tile([128, H * JF], F16, tag="AsE")
            nc.scalar.activation(
                out=_ap(AsE[:], 0, [[JF, H], [DE, FIN], [1, D_PAD]]),
                in_=_ap(AsAd[:], 0, [[FIN, H], [1, FIN], [0, D_PAD]]),
                func=ACT.Copy)
            nc.scalar.activation(
                out=_ap(AsE[:], D_PAD, [[JF, H], [DE, FIN]]),
                in_=_ap(AsAd[:], 28, [[FIN, H], [1, FIN]]),
                func=ACT.Copy)

            # linWp = lnw * linW (cols 7:14 of lin2 region, inside CP)
            nc.vector.tensor_scalar(
                out=CP[:, C_LIN + CLS : C_LIN + 2 * CLS],
                in0=CP[:, C_LIN : C_LIN + CLS],
                scalar1=CP[:, C_LNW : C_LNW + 1], scalar2=None, op0=OP.mult)
            # Wbc[32q+m, k] = (Wb@linWp')[m,k] (m<28) / (gb@linWp')[k] (m=28)
            ps_w = pp_p.tile([125, CLS], F32, tag="psw")
            nc.tensor.matmul(
                out=ps_w[:], lhsT=CP[:, C_WT : C_WT + 125],
                rhs=CP[:, C_LIN + CLS : C_LIN + 2 * CLS],
                start=True, stop=True)
            WbF = cp.tile([128, 136], F16, tag="WbF")
            nc.scalar.activation(out=WbF[:, 0:128],
                                 in_=CP[:, C_WB : C_WB + 128], func=ACT.Copy)
            nc.scalar.activation(out=WbF[0:125, 128:135], in_=ps_w[:],
                                 func=ACT.Copy)
            id16 = cp.tile([128, 128], F16, tag="id16")
            nc.scalar.activation(out=id16[:], in_=ident, func=ACT.Copy)
            # wsum col: -(row sum of [Wb;gb]) / HC  -> matmul emits -mean(o)
            ws = cp.tile([128, 1], F32, tag="ws")
            nc.vector.tensor_reduce(
                out=ws[:], in_=CP[:, C_WB : C_WB + 128], axis=AX, op=OP.add)
            nc.scalar.activation(out=WbF[:, 135:136], in_=ws[:],
                                 func=ACT.Copy, scale=-1.0 / HC)
            # [lbp | colsum] broadcast rows: lbp = lnb@linW + lb,
            # colsum = ones@linWp
            ps_a = pp_p.tile([2, 14], F32, tag="psa")
            nc.tensor.matmul(
                out=ps_a[:], lhsT=CP[:, C_LNB : C_LNB + 2],
                rhs=CP[:, C_LIN : C_LIN + 14],
                start=True, stop=True)
            z2a = cp.tile([2, 14], F32, tag="z2a")
            nc.vector.tensor_tensor(out=z2a[:], in0=ps_a[:],
                                    in1=CP[0:2, C_SEL : C_SEL + 14],
                                    op=OP.mult)
            z2 = cp.tile([2, 14], F32, tag="z2")
            nc.vector.tensor_tensor(out=z2[:], in0=z2a[:],
                                    in1=CP[0:2, C_LB2 : C_LB2 + 14],
                                    op=OP.add)
            ps_b = pp_p.tile([128, 14], F32, tag="psb")
            nc.tensor.matmul(
                out=ps_b[:], lhsT=CP[0:2, C_O2 : C_O2 + 128],
                rhs=z2[:], start=True, stop=True)
            LC = cp.tile([128, 14], F32, tag="LC")
            nc.scalar.copy(out=LC[:], in_=ps_b[:])

            # ---- main loop: 2 groups of G tiles
            for g in range(2):
                t0 = g * G
                xg = wp.tile([128, G * COLT], F32, tag="xg")
                nc.sync.dma_start(
                    out=xg[:], in_=d_xg[:, t0 * COLT : (t0 + G) * COLT])
                # bf16 cast (split for earlier pipeline start)
                xb = wp.tile([128, G * COLT], F16, tag="xb")
                h1 = (G // 2) * COLT
                nc.scalar.activation(out=xb[:, 0:h1], in_=xg[:, 0:h1],
                                     func=ACT.Copy)
                nc.scalar.activation(out=xb[:, h1:], in_=xg[:, h1:],
                                     func=ACT.Copy)

                # a_dst (from fp32 xg, small): ad[s,(t,h)] = sum_f xslot*Ad
                pd = wp.tile([128, G * H * FIN], F32, tag="pd")
                nc.vector.tensor_tensor(
                    out=_ap(pd[:], 0, [[H * FIN, G], [FIN, H], [1, FIN]]),
                    in0=_ap(xg[:], D_PAD, [[COLT, G], [0, H], [DE, FIN]]),
                    in1=_ap(AsAd[:], 28, [[0, G], [FIN, H], [1, FIN]]),
                    op=OP.mult)
                adt = wp.tile([128, G * H], F32, tag="adt")
                nc.vector.tensor_reduce(
                    out=_ap(adt[:], 0, [[H, G], [1, H]]),
                    in_=_ap(pd[:], 0, [[H * FIN, G], [FIN, H], [1, FIN]]),
                    axis=AX, op=OP.add)
                adb = wp.tile([128, G * H], F16, tag="adb")
                nc.scalar.activation(out=adb[:], in_=adt[:], func=ACT.Copy)
                # a_src: pa[s,(t,h,f,j)] = xb * AsE   (bf16 4x)
                pa = wp.tile([128, G * H * JF], F16, tag="pa")
                for t in range(G):
                    nc.vector.tensor_tensor(
                        out=_ap(pa[:], t * H * JF,
                                [[JF, H], [DE, FIN], [1, DE]]),
                        in0=_ap(xb[:], t * COLT,
                                [[0, H], [DE, FIN], [1, DE]]),
                        in1=_ap(AsE[:], 0, [[JF, H], [DE, FIN], [1, DE]]),
                        op=OP.mult)
                # tree-reduce over f (7 = 3+3+1): sE[s,(t,h,j)]
                TH = G * H
                q3 = wp.tile([128, TH * 3 * DE], F16, tag="q3")
                nc.vector.tensor_tensor(
                    out=_ap(q3[:], 0, [[3 * DE, TH], [DE, 3], [1, DE]]),
                    in0=_ap(pa[:], 0, [[JF, TH], [DE, 3], [1, DE]]),
                    in1=_ap(pa[:], 3 * DE, [[JF, TH], [DE, 3], [1, DE]]),
                    op=OP.add)
                r1 = wp.tile([128, TH * DE], F16, tag="r1")
                nc.vector.tensor_tensor(
                    out=_ap(r1[:], 0, [[DE, TH], [1, DE]]),
                    in0=_ap(q3[:], 0, [[3 * DE, TH], [1, DE]]),
                    in1=_ap(q3[:], DE, [[3 * DE, TH], [1, DE]]),
                    op=OP.add)
                r2 = wp.tile([128, TH * DE], F16, tag="r2")
                nc.vector.tensor_tensor(
                    out=_ap(r2[:], 0, [[DE, TH], [1, DE]]),
                    in0=_ap(r1[:], 0, [[DE, TH], [1, DE]]),
                    in1=_ap(q3[:], 2 * DE, [[3 * DE, TH], [1, DE]]),
                    op=OP.add)
                sE = wp.tile([128, TH * DE], F16, tag="sE")
                nc.vector.tensor_tensor(
                    out=_ap(sE[:], 0, [[DE, TH], [1, DE]]),
                    in0=_ap(r2[:], 0, [[DE, TH], [1, DE]]),
                    in1=_ap(pa[:], 6 * DE, [[JF, TH], [1, DE]]),
                    op=OP.add)
                # + mask (from fp32 xg), + a_dst bcast (gpsimd), leaky
                sF = wp.tile([128, TH * DE], F16, tag="sF")
                nc.vector.tensor_tensor(
                    out=_ap(sF[:], 0, [[JH, G], [DE, H], [1, DE]]),
                    in0=_ap(sE[:], 0, [[JH, G], [DE, H], [1, DE]]),
                    in1=_ap(xb[:], O_MK, [[COLT, G], [0, H], [1, DE]]),
                    op=OP.add)
                sG = wp.tile([128, TH * DE], F16, tag="sG")
                nc.gpsimd.tensor_tensor(
                    out=_ap(sG[:], 0, [[JH, G], [DE, H], [1, DE]]),
                    in0=_ap(sF[:], 0, [[JH, G], [DE, H], [1, DE]]),
                    in1=_ap(adb[:], 0, [[H, G], [1, H], [0, DE]]),
                    op=OP.add)
                ezl = wp.tile([128, TH * DE], F16, tag="ezl")
                nc.vector.scalar_tensor_tensor(
                    out=ezl[:], in0=sG[:], scalar=NEG, in1=sG[:],
                    op0=OP.mult, op1=OP.max)
                mx = wp.tile([128, G * H], F16, tag="mx")
                nc.vector.tensor_reduce(
                    out=_ap(mx[:], 0, [[H, G], [1, H]]),
                    in_=_ap(ezl[:], 0, [[JH, G], [DE, H], [1, DE]]),
                    axis=AX, op=OP.max)
                ezm = wp.tile([128, TH * DE], F16, tag="ezm")
                nc.gpsimd.tensor_tensor(
                    out=_ap(ezm[:], 0, [[JH, G], [DE, H], [1, DE]]),
                    in0=_ap(ezl[:], 0, [[JH, G], [DE, H], [1, DE]]),
                    in1=_ap(mx[:], 0, [[H, G], [1, H], [0, DE]]),
                    op=OP.subtract)
                ez = wp.tile([128, TH * DE], F16, tag="ez")
                nc.scalar.activation(out=ez[:], in_=ezm[:], func=ACT.Exp)

                # denominators (fp32)
                den = wp.tile([128, G * H], F32, tag="den")
                nc.vector.tensor_reduce(
                    out=_ap(den[:], 0, [[H, G], [1, H]]),
                    in_=_ap(ez[:], 0, [[JH, G], [DE, H], [1, DE]]),
                    axis=AX, op=OP.add)

                # messages: pm[s,(t,h,f,j)] = ez * xb  (bf16 4x, j<D_PAD)
                pm = wp.tile([128, TH * MJF], F16, tag="pm")
                for t in range(G):
                    nc.vector.tensor_tensor(
                        out=_ap(pm[:], t * H * MJF,
                                [[MJF, H], [D_PAD, FIN], [1, D_PAD]]),
                        in0=_ap(ez[:], t * JH,
                                [[DE, H], [0, FIN], [1, D_PAD]]),
                        in1=_ap(xb[:], t * COLT,
                                [[0, H], [DE, FIN], [1, D_PAD]]),
                        op=OP.mult)
                # generic pairwise tree-reduce over j; odd leftovers are
                # folded in at the end; small levels accumulate in fp32
                THF = TH * FIN
                cur, stride, ncur = pm, D_PAD, D_PAD
                leftovers = []
                lvl = 0
                while ncur > 1:
                    half = ncur // 2
                    if ncur % 2:
                        leftovers.append((cur, stride, ncur - 1))
                    dt = F16 if half >= 4 else F32
                    nxt = wp.tile([128, THF * half], dt, tag=f"jt{lvl}")
                    nc.vector.tensor_tensor(
                        out=_ap(nxt[:], 0, [[half, THF], [1, half]]),
                        in0=_ap(cur[:], 0, [[stride, THF], [1, half]]),
                        in1=_ap(cur[:], half, [[stride, THF], [1, half]]),
                        op=OP.add)
                    cur, stride, ncur = nxt, half, half
                    lvl += 1
                for li, (buf, st, off) in enumerate(leftovers):
                    nxt = wp.tile([128, THF], F32, tag=f"jl{li}")
                    nc.vector.tensor_tensor(
                        out=_ap(nxt[:], 0, [[1, THF]]),
                        in0=_ap(cur[:], 0, [[stride, THF]]),
                        in1=_ap(buf[:], off, [[st, THF]]),
                        op=OP.add)
                    cur, stride = nxt, 1
                SD = cur

                # normalize: Sn = SD/(den+eps), ones col 28 per tile
                rd = wp.tile([128, G * H], F32, tag="rd")
                nc.scalar.activation(out=rd[:], in_=den[:], func=ACT.Copy,
                                     bias=1e-16)
                nc.vector.reciprocal(out=rd[:], in_=rd[:])
                Sn = wp.tile([128, G * 32], F16, tag="Sn")
                nc.gpsimd.memset(_ap(Sn[:], 28, [[32, G]]), 1.0)
                nc.vector.tensor_tensor(
                    out=_ap(Sn[:], 0, [[32, G], [FIN, H], [1, FIN]]),
                    in0=_ap(SD[:], 0, [[H * FIN, G], [FIN, H], [1, FIN]]),
                    in1=_ap(rd[:], 0, [[H, G], [1, H], [0, FIN]]),
                    op=OP.mult)

                # transpose 4 tiles per PE pass; quadrant-aligned SnT
                SnTs = []
                for b in range(NB):
                    w = min(96, G * 32 - b * 96)
                    ps_t = pp_t.tile([96, 128], F16, tag="pst")
                    nc.tensor.transpose(out=ps_t[0:w, :],
                                        in_=Sn[:, b * 96 : b * 96 + w],
                                        identity=id16)
                    SnT = wp.tile([96, 128], F16, tag=f"snt{b}")
                    nc.scalar.activation(out=SnT[0:w, :], in_=ps_t[0:w, :],
                                         func=ACT.Copy)
                    SnTs.append(SnT)

                # per tile: [o | q | -mu] = SnT.T @ WbFull, then LN + logits
                vs = wp.tile([128, G], F32, tag="vs")
                nm = wp.tile([128, G], F32, tag="nm")
                u_sb = wp.tile([128, G * CLS], F32, tag="u")
                lg = wp.tile([128, G * CLS], F32, tag="lg")
                for t in range(G):
                    b, tb = divmod(t, 3)
                    base = tb * 32
                    ps_o = pp_o.tile([128, 136], F32, tag="pso")
                    nc.tensor.matmul(
                        out=ps_o[:],
                        lhsT=SnTs[b][base : base + 29, :],
                        rhs=WbF[base : base + 29, :],
                        start=True, stop=True)
                    nc.vector.tensor_scalar(
                        out=nm[:, t : t + 1], in0=ps_o[:, 135:136],
                        scalar1=1.0, scalar2=None, op0=OP.mult)
                    sqt = wp.tile([128, HC], F32, tag="sqt")
                    nc.scalar.activation(
                        out=sqt[:], in_=ps_o[:, 0:HC], func=ACT.Square,
                        bias=nm[:, t : t + 1], accum_out=vs[:, t : t + 1])
                    nc.vector.scalar_tensor_tensor(
                        out=u_sb[:, t * CLS : (t + 1) * CLS],
                        in0=LC[:, CLS:14], scalar=ps_o[:, 135:136],
                        in1=ps_o[:, HC : HC + CLS],
                        op0=OP.mult, op1=OP.add)
                nc.scalar.activation(out=vs[:], in_=vs[:], func=ACT.Ln,
                                     scale=1.0 / HC,
                                     bias=CP[:, C_EPS : C_EPS + 1])
                nc.scalar.activation(out=vs[:], in_=vs[:], func=ACT.Exp,
                                     scale=-0.5)
                for t in range(G):
                    nc.vector.scalar_tensor_tensor(
                        out=lg[:, t * CLS : (t + 1) * CLS],
                        in0=u_sb[:, t * CLS : (t + 1) * CLS],
                        scalar=vs[:, t : t + 1], in1=LC[:, 0:CLS],
                        op0=OP.mult, op1=OP.add)
                nc.scalar.activation(out=lg[:], in_=lg[:], func=ACT.Exp)
                se = wp.tile([128, G], F32, tag="se")
                nc.vector.tensor_reduce(
                    out=se[:], in_=lg[:].rearrange("p (t e) -> p t e", t=G),
                    axis=AX, op=OP.add)
                nc.vector.reciprocal(out=se[:], in_=se[:])
                po = wp.tile([128, G * CLS], F32, tag="po")
                nc.gpsimd.tensor_tensor(
                    out=po[:],
                    in0=lg[:].rearrange("p (t e) -> p t e", t=G),
                    in1=_ap(se[:], 0, [[1, G], [0, CLS]]), op=OP.mult)
                nc.sync.dma_start(
                    out=d_out[:, t0 * CLS : (t0 + G) * CLS], in_=po[:])

    nc.compile()
    return nc


_CACHE = {}


def _program(T_pc, D_PAD, COLT):
    key = (T_pc, D_PAD, COLT)
    if key not in _CACHE:
        _CACHE[key] = _build(T_pc, D_PAD, COLT)
    return _CACHE[key]


# ---------------------------------------------------------------- entry
def kernel(x, edge_weight, W, att_src, att_dst, gat_bias, ln_w, ln_b,
           lin_W, lin_b, edge_index, ids):
    prep = _preprocess(np.asarray(x), np.asarray(edge_index),
                       np.asarray(ids))
    T_pc, D_PAD, COLT = prep["T_pc"], prep["D_PAD"], prep["COLT"]
    nc = _program(T_pc, D_PAD, COLT)
    cpack = _const_pack(W, att_src, att_dst, gat_bias, ln_w, ln_b,
                        lin_W, lin_b)

    in_maps = [{"xg2": prep["xg2"][c], "cpack": cpack}
               for c in range(NCORES)]

    if os.environ.get("KERNEL_SIM"):
        from concourse.bass_interp import CoreSim

        outs = []
        ncores = int(os.environ.get("KERNEL_SIM_CORES", "1"))
        for c in range(ncores):
            sim = CoreSim(nc, require_finite=False, require_nnan=False)
            for k, v in in_maps[c].items():
                sim.tensor(k)[:] = v
            sim.simulate()
            outs.append(sim.tensor("probs").copy())
        arr = np.stack(outs + [np.zeros_like(outs[0])] * (NCORES - ncores))
    else:
        trace = bool(int(os.environ.get("KERNEL_TRACE", "0")))
        res = bass_utils.run_bass_kernel_spmd(
            nc, in_maps, core_ids=list(range(NCORES)), trace=trace)
        if trace and res.exec_time_ns is not None:
            print(f"HW exec time: {res.exec_time_ns} ns")
        arr = np.stack([res.results[c]["probs"] for c in range(NCORES)])

    full = (arr.reshape(NCORES, 128, T_pc, CLS)
            .transpose(0, 2, 1, 3)
            .reshape(NCORES * T_pc * 128, CLS))
    return np.ascontiguousarray(full[prep["inv"]], np.float32)
